# revision 65
# baseline (speedup 1.0000x reference)
"""DenseGAT Trainium2 kernel (8 NeuronCores, batch-parallel).

Math per (batch, head):
  h = x @ W.T ; a_src[i] = h[i]*att_src ; a_dst[j] = h[j]*att_dst
  s_ij = a_src[i] + a_dst[j] ; P = adj * exp(leakyrelu_0.2(s))
  out[i] = (P @ h)[i] / sum_j P[i,j]

Identity: exp(lrelu_0.2(s)) = p_i * q_j * max(1, u_i v_j)
with u = exp(0.8 a_src), v = exp(0.8 a_dst), q = exp(0.2 a_dst); p_i
cancels in the softmax ratio. Fold q into the j-side tensor_scalar:
  t1'[j,i] = q_j * max(1, u_i v_j) = max(u_i * e^{b_j}, e^{0.2 b_j})
(one DVE tensor_scalar at 4x: op0=mult scalar1=e^{b_j}, op1=max
 scalar2=e^{0.2 b_j}, per-partition vectors), then
  AM[j,i] = t1'[j,i] * adjT[j,i]     (DVE tensor_tensor at 2x,
                                      batched 4 j-tiles per instruction)
  out_aug[i,:] = sum_j AM[j,i] * [1 | h_j]   (PE, accumulated over j)
  out = out_aug[:,1:65] / out_aug[:,0]

Host pre-transposes layouts (adjT as bf16 {0,1}, xT, W/WT bf16, attW
assembled) so the device spends nothing on transposition/upcasting.
gpsimd tensor_tensor is ruled out: a concurrent gpsimd TT degrades DVE
tensor_scalar from 4x to ~1x via the shared SBUF ports (measured).
Each core handles one batch sample.
"""

import numpy as np
import ml_dtypes

import concourse.bass as bass
import concourse.mybir as mybir
import concourse.tile as tile
from concourse import bacc
from concourse.bass_utils import run_bass_kernel_spmd
from concourse.masks import make_identity

P = 128
B, L, CIN, COUT, HEADS = 8, 2048, 256, 256, 4
HD = COUT // HEADS          # 64
NT = L // P                 # 16 tiles along L
KB = CIN // P               # 2 chunks along cin/cout
NAUG = HD + 1               # 65 (col 0 = softmax denominator)
N_CORES = 8
PK = 4                      # j-tiles per batched tensor_tensor
NPK = NT // PK

F32 = mybir.dt.float32
BF16 = mybir.dt.bfloat16
AF = mybir.ActivationFunctionType
OP = mybir.AluOpType

_NC_CACHE = {}


def _build():
    nc = bacc.Bacc(None, target_bir_lowering=False, debug=False)
    adjT_in = nc.declare_dram_parameter("adjT", [L, L], BF16, isOutput=False)
    xT_in = nc.declare_dram_parameter("xT", [CIN, L], BF16, isOutput=False)
    w_in = nc.declare_dram_parameter("W", [COUT, CIN], BF16, isOutput=False)
    wT_in = nc.declare_dram_parameter("WT", [CIN, COUT], BF16, isOutput=False)
    attw_in = nc.declare_dram_parameter("attW", [COUT, 2 * HEADS], BF16, isOutput=False)
    out_d = nc.declare_dram_parameter("out", [L, COUT], F32, isOutput=True)
    urows_d = nc.dram_tensor("urows_scratch", [4, L], BF16, kind="Internal")

    with tile.TileContext(nc) as tc:
        with (
            tc.tile_pool(name="const", bufs=1) as cpool,
            tc.tile_pool(name="big", bufs=1) as big,
            tc.tile_pool(name="t1p", bufs=2) as t1p,
            tc.tile_pool(name="amp", bufs=2) as amp,
            tc.tile_pool(name="outst", bufs=2) as outp,
        ):
            ident_bf = cpool.tile([P, P], BF16)
            make_identity(nc, ident_bf)


            # persistent tensors
            adjT = big.tile([P, NT, L], BF16)      # adjT[j%128, j//128, i]
            # x^T (cin on partitions), split [kb, L-half, col] so each DMA
            # writes one full dim-1/2 slot (sub-tile dep granularity)
            xT_bf = big.tile([P, KB, 2, L // 2], BF16)
            wT_bf = big.tile([P, KB, COUT], BF16)  # W^T (cin on partitions)
            w_nat = big.tile([P, KB, CIN], BF16)   # W natural (cout on part)
            haug = big.tile([P, NT, HEADS * NAUG], BF16)  # [1|h0][1|h1][1|h2][1|h3]
            urows = big.tile([4, L], BF16)         # exp(0.8 a_src_h) rows
            evq = big.tile([4, L], BF16)           # e^{b_h} rows
            eq = big.tile([4, L], BF16)            # e^{0.2 b_h} rows
            ecols = big.tile([P, NT, 8], F32)      # transposed: vq_h | q_h columns
            ubc = big.tile([P, HEADS, L], BF16)    # exp(0.8 a_src_h) bcast, all heads

            # ---------------- prep ----------------
            with (
                tc.tile_pool(name="big2", bufs=1) as big2,
                tc.tile_pool(name="h_ps", bufs=2, space="PSUM") as hps,
                tc.tile_pool(name="sc_ps", bufs=1, space="PSUM") as scps,
                tc.tile_pool(name="tr_ps", bufs=2, space="PSUM") as trps,
            ):
                attW = big2.tile([P, KB, 2 * HEADS], BF16)     # [cout, 8]
                attc_bf = big2.tile([P, KB, 2 * HEADS], BF16)  # [cin, 8]

                # DMAs. Two hardware queues: sync gets xT's first L-half
                # (both cin chunks — unblocks scores/h for L-half 0 early)
                # then the adjT stream; scalar(Activation) gets the small
                # weights + xT's second L-half, later urows/ubcast.
                LH = L // 2
                nc.sync.dma_start(
                    out=xT_bf[:, 0, 0, :], in_=xT_in[0:P, 0:LH]
                )
                nc.sync.dma_start(
                    out=xT_bf[:, 1, 0, :], in_=xT_in[P : 2 * P, 0:LH]
                )
                nc.scalar.dma_start(
                    out=attW[:], in_=attw_in[:].rearrange("(kb p) c -> p kb c", p=P)
                )
                nc.scalar.dma_start(
                    out=w_nat[:], in_=w_in[:].rearrange("(kb p) c -> p kb c", p=P)
                )
                nc.scalar.dma_start(
                    out=xT_bf[:, 0, 1, :], in_=xT_in[0:P, LH:L]
                )
                nc.scalar.dma_start(
                    out=xT_bf[:, 1, 1, :], in_=xT_in[P : 2 * P, LH:L]
                )
                nc.scalar.dma_start(
                    out=wT_bf[:], in_=wT_in[:].rearrange("(kb p) o -> p kb o", p=P)
                )
                for t in range(NT):
                    nc.sync.dma_start(
                        out=adjT[:, t, :], in_=adjT_in[t * P : (t + 1) * P, :]
                    )

                # attc = W^T @ attW  (tiny)
                for mb in range(KB):
                    ap_ps = trps.tile([P, 2 * HEADS], F32, tag="tr")
                    for cb in range(KB):
                        nc.tensor.matmul(
                            ap_ps[:], w_nat[:, cb, mb * P : (mb + 1) * P], attW[:, cb, :],
                            start=(cb == 0), stop=(cb == KB - 1),
                        )
                    nc.scalar.activation(attc_bf[:, mb, :], ap_ps[:], AF.Copy, bias=0.0, scale=1.0)

                # scores: src rows and dst rows in separate PSUM tiles
                # (PSUM reads must start at partition 0), split in L-halves
                # to fit banks; exps applied straight from PSUM.
                #   urows = e^{0.8 a_src}; erows = [e^{b} | e^{0.2 b}]
                for h2 in range(2):
                    sl = slice(h2 * 1024, (h2 + 1) * 1024)
                    asrc_ps = scps.tile([4, 2, 512], F32, tag="sc_src")
                    adst_ps = scps.tile([4, 2, 512], F32, tag="sc_dst")
                    for nb in range(2):
                        for kb in range(KB):
                            nc.tensor.matmul(
                                asrc_ps[:, nb, :], attc_bf[:, kb, 0:HEADS],
                                xT_bf[:, kb, h2, nb * 512 : (nb + 1) * 512],
                                start=(kb == 0), stop=(kb == KB - 1),
                            )
                    for nb in range(2):
                        for kb in range(KB):
                            nc.tensor.matmul(
                                adst_ps[:, nb, :], attc_bf[:, kb, HEADS : 2 * HEADS],
                                xT_bf[:, kb, h2, nb * 512 : (nb + 1) * 512],
                                start=(kb == 0), stop=(kb == KB - 1),
                            )
                    nc.scalar.activation(
                        urows[:, sl].rearrange("p (nb c) -> p nb c", c=512),
                        asrc_ps[:], AF.Exp, bias=0.0, scale=0.8,
                    )
                    nc.scalar.activation(
                        evq[:, sl].rearrange("p (nb c) -> p nb c", c=512),
                        adst_ps[:], AF.Exp, bias=0.0, scale=1.0,
                    )
                    nc.scalar.activation(
                        eq[:, sl].rearrange("p (nb c) -> p nb c", c=512),
                        adst_ps[:], AF.Exp, bias=0.0, scale=0.2,
                    )
                nc.scalar.dma_start(out=urows_d[:], in_=urows[:])

                # transposed per-j columns: ecols[:,t,0:4]=vq, 4:8=q
                for t in range(NT):
                    ecp = trps.tile([P, 8], BF16, tag="tr")
                    nc.tensor.transpose(
                        ecp[:, 0:4], evq[:, t * P : (t + 1) * P], ident_bf[0:4, 0:4]
                    )
                    nc.tensor.transpose(
                        ecp[:, 4:8], eq[:, t * P : (t + 1) * P], ident_bf[0:4, 0:4]
                    )
                    nc.vector.tensor_copy(ecols[:, t, :], ecp[:])

                # ubcast for all heads upfront (overlaps head-0 compute)
                for h in range(HEADS):
                    nc.scalar.dma_start(
                        out=ubc[:, h, :], in_=urows_d[h : h + 1, :].to_broadcast((P, L))
                    )

                # haug = [1 | h] per head; ones first so h-evac can overwrite
                for h in range(HEADS):
                    nc.vector.memset(
                        haug[:, :, h * NAUG : h * NAUG + 1].rearrange("p t one -> p (t one)"),
                        1.0,
                    )
                # h = x @ W.T, written strided into the 4 head slots
                for c in range(NT):
                    hp = hps.tile([P, COUT], F32, tag="hp")
                    for kb in range(KB):
                        nc.tensor.matmul(
                            hp[:], xT_bf[:, kb, c // 8, (c % 8) * P : (c % 8 + 1) * P],
                            wT_bf[:, kb, :],
                            start=(kb == 0), stop=(kb == KB - 1),
                        )
                    dst = haug[:, c, :].rearrange("p (g q) -> p g q", q=NAUG)[:, :, 1:NAUG]
                    nc.scalar.activation(
                        dst, hp[:].rearrange("p (g q) -> p g q", q=HD),
                        AF.Copy, bias=0.0, scale=1.0,
                    )

            # ---------------- per-head attention ----------------
            with tc.tile_pool(name="mm_ps", bufs=8, space="PSUM") as mmps:

                def out_tail(poq, h):
                    rall = cpool.tile([P, NT], F32, tag="rall", name="rall")
                    out_stage = outp.tile([P, NT, HD], F32, tag="outst", name="outst")
                    out_view = out_d[:].rearrange("(c p) (hh d) -> p c hh d", p=P, d=HD)
                    for qd in range(4):
                        nc.vector.reciprocal(
                            rall[:, qd * 4 : (qd + 1) * 4],
                            poq[qd][:, :, 0:1].rearrange("p c one -> p (c one)"),
                        )
                        for cgm in range(4):
                            cg = qd * 4 + cgm
                            if h == HEADS - 1:
                                nc.vector.tensor_scalar(
                                    out=out_stage[:, cg, :], in0=poq[qd][:, cgm, 1:NAUG],
                                    scalar1=rall[:, cg : cg + 1], scalar2=None,
                                    op0=OP.mult,
                                )
                            else:
                                nc.scalar.activation(
                                    out_stage[:, cg, :], poq[qd][:, cgm, 1:NAUG],
                                    AF.Identity, bias=0.0, scale=rall[:, cg : cg + 1],
                                )
                        nc.sync.dma_start(
                            out=out_view[:, qd * 4 : (qd + 1) * 4, h, :],
                            in_=out_stage[:, qd * 4 : (qd + 1) * 4, :],
                        )

                for h in range(HEADS):
                    poq = [
                        mmps.tile([P, 4, NAUG], F32, tag="poq", name="poq") for _ in range(4)
                    ]
                    rhs = haug[:, :, h * NAUG : (h + 1) * NAUG]
                    for pk in range(NPK):
                        # t1 = max(u_i * vq_j, q_j) per tile; AM = t1 * adjT
                        # batched over PK j-tiles per tensor_tensor
                        t1 = t1p.tile([P, PK, L], BF16, tag="t1", name="t1")
                        for k in range(PK):
                            t = pk * PK + k
                            nc.vector.tensor_scalar(
                                out=t1[:, k, :], in0=ubc[:, h, :],
                                scalar1=ecols[:, t, h : h + 1],
                                scalar2=ecols[:, t, HEADS + h : HEADS + h + 1],
                                op0=OP.mult, op1=OP.max,
                            )
                        am = amp.tile([P, PK, L], BF16, tag="am", name="am")
                        nc.vector.tensor_tensor(
                            out=am[:], in0=t1[:], in1=adjT[:, pk * PK : (pk + 1) * PK, :],
                            op=OP.mult,
                        )
                        for k in range(PK):
                            t = pk * PK + k
                            for cg in range(NT):
                                # start only on the first slice of each quad:
                                # pending-zero covers the whole PSUM bank.
                                nc.tensor.matmul(
                                    poq[cg // 4][:, cg % 4, :],
                                    am[:, k, cg * P : (cg + 1) * P], rhs[:, t, :],
                                    start=(t == 0 and cg % 4 == 0), stop=(t == NT - 1),
                                    skip_group_check=True,
                                )
                    out_tail(poq, h)

    nc.finalize()
    return nc


_LUT_BF16_01 = np.array([0x0000, 0x3F80], dtype=np.uint16)  # {0.0, 1.0} in bf16


def _prep_in_maps(x, adj_mask, W, att_src, att_dst):
    x = np.asarray(x, dtype=np.float32)
    W = np.asarray(W, dtype=np.float32)
    att_src = np.asarray(att_src, dtype=np.float32)
    att_dst = np.asarray(att_dst, dtype=np.float32)
    adj_u8 = np.asarray(adj_mask).view(np.uint8)

    w_bf = np.ascontiguousarray(W.astype(ml_dtypes.bfloat16))
    wT_bf = np.ascontiguousarray(W.T.astype(ml_dtypes.bfloat16))
    # attW layout: col h = att_src_h, col 4+h = att_dst_h, rows = cout
    attw = np.zeros((COUT, 2 * HEADS), dtype=np.float32)
    for h in range(HEADS):
        attw[HD * h : HD * (h + 1), h] = att_src[0, h, 0, :]
        attw[HD * h : HD * (h + 1), HEADS + h] = att_dst[0, h, 0, :]
    attw_bf = np.ascontiguousarray(attw.astype(ml_dtypes.bfloat16))

    in_maps = []
    for b in range(N_CORES):
        adjT_bf = np.ascontiguousarray(
            _LUT_BF16_01[adj_u8[b].T]
        ).view(ml_dtypes.bfloat16)
        xT_bf = np.ascontiguousarray(x[b].T.astype(ml_dtypes.bfloat16))
        in_maps.append(
            {
                "adjT": adjT_bf,
                "xT": xT_bf,
                "W": w_bf,
                "WT": wT_bf,
                "attW": attw_bf,
            }
        )
    return in_maps


def kernel(x, adj_mask, W, att_src, att_dst):
    if "nc" not in _NC_CACHE:
        _NC_CACHE["nc"] = _build()
    nc = _NC_CACHE["nc"]
    in_maps = _prep_in_maps(x, adj_mask, W, att_src, att_dst)
    res = run_bass_kernel_spmd(nc, in_maps, core_ids=list(range(N_CORES)))
    out = np.stack([res.results[b]["out"] for b in range(N_CORES)], axis=0)
    return out.astype(np.float32)


# revision 69
# speedup vs baseline: 1.0511x; 1.0511x over previous
"""DenseGAT Trainium2 kernel (8 NeuronCores, batch-parallel).

Math per (batch, head):
  h = x @ W.T ; a_src[i] = h[i]*att_src ; a_dst[j] = h[j]*att_dst
  s_ij = a_src[i] + a_dst[j] ; P = adj * exp(leakyrelu_0.2(s))
  out[i] = (P @ h)[i] / sum_j P[i,j]

Identity: exp(lrelu_0.2(s)) = p_i * q_j * max(1, u_i v_j)
with u = exp(0.8 a_src), v = exp(0.8 a_dst), q = exp(0.2 a_dst); p_i
cancels in the softmax ratio. Fold q into the j-side tensor_scalar:
  t1'[j,i] = q_j * max(1, u_i v_j) = max(u_i * e^{b_j}, e^{0.2 b_j})
(one DVE tensor_scalar at 4x: op0=mult scalar1=e^{b_j}, op1=max
 scalar2=e^{0.2 b_j}, per-partition vectors), then
  AM[j,i] = t1'[j,i] * adjT[j,i]     (DVE tensor_tensor at 2x,
                                      batched 4 j-tiles per instruction)
  out_aug[i,:] = sum_j AM[j,i] * [1 | h_j]   (PE, accumulated over j)
  out = out_aug[:,1:65] / out_aug[:,0]

Host pre-transposes layouts (adjT as bf16 {0,1}, xT, W/WT bf16, attW
assembled) so the device spends nothing on transposition/upcasting.
gpsimd tensor_tensor is ruled out: a concurrent gpsimd TT degrades DVE
tensor_scalar from 4x to ~1x via the shared SBUF ports (measured).
Each core handles one batch sample.
"""

import numpy as np
import ml_dtypes

import concourse.bass as bass
import concourse.mybir as mybir
import concourse.tile as tile
from concourse import bacc
from concourse.bass_utils import run_bass_kernel_spmd
from concourse.masks import make_identity

P = 128
B, L, CIN, COUT, HEADS = 8, 2048, 256, 256, 4
HD = COUT // HEADS          # 64
NT = L // P                 # 16 tiles along L
KB = CIN // P               # 2 chunks along cin/cout
NAUG = HD + 1               # 65 (col 0 = softmax denominator)
N_CORES = 8
PK = 4                      # j-tiles per batched tensor_tensor
NPK = NT // PK
# tiles whose max() runs on the Activation engine via q+relu(a-q); must
# avoid pack 0 (start flags) and pack 3 (extra MMs would follow the stop)
ACT_SET = {4, 6, 9, 11}

F32 = mybir.dt.float32
BF16 = mybir.dt.bfloat16
AF = mybir.ActivationFunctionType
OP = mybir.AluOpType

_NC_CACHE = {}


def _build():
    nc = bacc.Bacc(None, target_bir_lowering=False, debug=False)
    adjT_in = nc.declare_dram_parameter("adjT", [L, L], BF16, isOutput=False)
    xT_in = nc.declare_dram_parameter("xT", [CIN, L], BF16, isOutput=False)
    w_in = nc.declare_dram_parameter("W", [COUT, CIN], BF16, isOutput=False)
    wT_in = nc.declare_dram_parameter("WT", [CIN, COUT], BF16, isOutput=False)
    attw_in = nc.declare_dram_parameter("attW", [COUT, 2 * HEADS], BF16, isOutput=False)
    out_d = nc.declare_dram_parameter("out", [L, COUT], F32, isOutput=True)
    urows_d = nc.dram_tensor("urows_scratch", [4, L], BF16, kind="Internal")

    with tile.TileContext(nc) as tc:
        with (
            tc.tile_pool(name="const", bufs=1) as cpool,
            tc.tile_pool(name="big", bufs=1) as big,
            tc.tile_pool(name="t1p", bufs=2) as t1p,
            tc.tile_pool(name="amp", bufs=2) as amp,
            tc.tile_pool(name="outst", bufs=2) as outp,
        ):
            ident_bf = cpool.tile([P, P], BF16)
            make_identity(nc, ident_bf)


            # persistent tensors
            adjT = big.tile([P, NT, L], BF16)      # adjT[j%128, j//128, i]
            # x^T (cin on partitions), split [kb, L-half, col] so each DMA
            # writes one full dim-1/2 slot (sub-tile dep granularity)
            xT_bf = big.tile([P, KB, 2, L // 2], BF16)
            wT_bf = big.tile([P, KB, COUT], BF16)  # W^T (cin on partitions)
            w_nat = big.tile([P, KB, CIN], BF16)   # W natural (cout on part)
            haug = big.tile([P, NT, HEADS * NAUG], BF16)  # [1|h0][1|h1][1|h2][1|h3]
            urows = big.tile([4, L], BF16)         # exp(0.8 a_src_h) rows
            evq = big.tile([4, L], BF16)           # e^{b_h} rows
            eq = big.tile([4, L], BF16)            # e^{0.2 b_h} rows
            ecols = big.tile([P, NT, 8], F32)      # transposed: vq_h | q_h columns
            nqcols = big.tile([P, NT, 4], F32)     # -q_h columns (ACT relu bias)
            ubc = big.tile([P, HEADS, L], BF16)    # exp(0.8 a_src_h) bcast, all heads

            # ---------------- prep ----------------
            with (
                tc.tile_pool(name="big2", bufs=1) as big2,
                tc.tile_pool(name="h_ps", bufs=2, space="PSUM") as hps,
                tc.tile_pool(name="sc_ps", bufs=1, space="PSUM") as scps,
                tc.tile_pool(name="tr_ps", bufs=2, space="PSUM") as trps,
            ):
                attW = big2.tile([P, KB, 2 * HEADS], BF16)     # [cout, 8]
                attc_bf = big2.tile([P, KB, 2 * HEADS], BF16)  # [cin, 8]

                # DMAs. Two hardware queues: sync gets xT's first L-half
                # (both cin chunks — unblocks scores/h for L-half 0 early)
                # then the adjT stream; scalar(Activation) gets the small
                # weights + xT's second L-half, later urows/ubcast.
                LH = L // 2
                nc.sync.dma_start(
                    out=xT_bf[:, 0, 0, :], in_=xT_in[0:P, 0:LH]
                )
                nc.sync.dma_start(
                    out=xT_bf[:, 1, 0, :], in_=xT_in[P : 2 * P, 0:LH]
                )
                nc.scalar.dma_start(
                    out=attW[:], in_=attw_in[:].rearrange("(kb p) c -> p kb c", p=P)
                )
                nc.scalar.dma_start(
                    out=w_nat[:], in_=w_in[:].rearrange("(kb p) c -> p kb c", p=P)
                )
                nc.scalar.dma_start(
                    out=xT_bf[:, 0, 1, :], in_=xT_in[0:P, LH:L]
                )
                nc.scalar.dma_start(
                    out=xT_bf[:, 1, 1, :], in_=xT_in[P : 2 * P, LH:L]
                )
                nc.scalar.dma_start(
                    out=wT_bf[:], in_=wT_in[:].rearrange("(kb p) o -> p kb o", p=P)
                )
                for t in range(NT):
                    nc.sync.dma_start(
                        out=adjT[:, t, :], in_=adjT_in[t * P : (t + 1) * P, :]
                    )

                # attc = W^T @ attW  (tiny)
                for mb in range(KB):
                    ap_ps = trps.tile([P, 2 * HEADS], F32, tag="tr")
                    for cb in range(KB):
                        nc.tensor.matmul(
                            ap_ps[:], w_nat[:, cb, mb * P : (mb + 1) * P], attW[:, cb, :],
                            start=(cb == 0), stop=(cb == KB - 1),
                        )
                    nc.scalar.activation(attc_bf[:, mb, :], ap_ps[:], AF.Copy, bias=0.0, scale=1.0)

                # scores: src rows and dst rows in separate PSUM tiles
                # (PSUM reads must start at partition 0), split in L-halves
                # to fit banks; exps applied straight from PSUM.
                #   urows = e^{0.8 a_src}; erows = [e^{b} | e^{0.2 b}]
                for h2 in range(2):
                    sl = slice(h2 * 1024, (h2 + 1) * 1024)
                    asrc_ps = scps.tile([4, 2, 512], F32, tag="sc_src")
                    adst_ps = scps.tile([4, 2, 512], F32, tag="sc_dst")
                    for nb in range(2):
                        for kb in range(KB):
                            nc.tensor.matmul(
                                asrc_ps[:, nb, :], attc_bf[:, kb, 0:HEADS],
                                xT_bf[:, kb, h2, nb * 512 : (nb + 1) * 512],
                                start=(kb == 0), stop=(kb == KB - 1),
                            )
                    for nb in range(2):
                        for kb in range(KB):
                            nc.tensor.matmul(
                                adst_ps[:, nb, :], attc_bf[:, kb, HEADS : 2 * HEADS],
                                xT_bf[:, kb, h2, nb * 512 : (nb + 1) * 512],
                                start=(kb == 0), stop=(kb == KB - 1),
                            )
                    nc.scalar.activation(
                        urows[:, sl].rearrange("p (nb c) -> p nb c", c=512),
                        asrc_ps[:], AF.Exp, bias=0.0, scale=0.8,
                    )
                    nc.scalar.activation(
                        evq[:, sl].rearrange("p (nb c) -> p nb c", c=512),
                        adst_ps[:], AF.Exp, bias=0.0, scale=1.0,
                    )
                    nc.scalar.activation(
                        eq[:, sl].rearrange("p (nb c) -> p nb c", c=512),
                        adst_ps[:], AF.Exp, bias=0.0, scale=0.2,
                    )
                nc.scalar.dma_start(out=urows_d[:], in_=urows[:])

                # transposed per-j columns: ecols[:,t,0:4]=vq, 4:8=q
                for t in range(NT):
                    ecp = trps.tile([P, 8], BF16, tag="tr")
                    nc.tensor.transpose(
                        ecp[:, 0:4], evq[:, t * P : (t + 1) * P], ident_bf[0:4, 0:4]
                    )
                    nc.tensor.transpose(
                        ecp[:, 4:8], eq[:, t * P : (t + 1) * P], ident_bf[0:4, 0:4]
                    )
                    nc.vector.tensor_copy(ecols[:, t, :], ecp[:])
                nc.scalar.activation(
                    nqcols[:], ecols[:, :, 4:8], AF.Copy, bias=0.0, scale=-1.0
                )

                # ubcast for all heads upfront (overlaps head-0 compute)
                for h in range(HEADS):
                    nc.scalar.dma_start(
                        out=ubc[:, h, :], in_=urows_d[h : h + 1, :].to_broadcast((P, L))
                    )

                # haug = [1 | h] per head; ones first so h-evac can overwrite
                for h in range(HEADS):
                    nc.vector.memset(
                        haug[:, :, h * NAUG : h * NAUG + 1].rearrange("p t one -> p (t one)"),
                        1.0,
                    )
                # h = x @ W.T, written strided into the 4 head slots
                for c in range(NT):
                    hp = hps.tile([P, COUT], F32, tag="hp")
                    for kb in range(KB):
                        nc.tensor.matmul(
                            hp[:], xT_bf[:, kb, c // 8, (c % 8) * P : (c % 8 + 1) * P],
                            wT_bf[:, kb, :],
                            start=(kb == 0), stop=(kb == KB - 1),
                        )
                    dst = haug[:, c, :].rearrange("p (g q) -> p g q", q=NAUG)[:, :, 1:NAUG]
                    nc.scalar.activation(
                        dst, hp[:].rearrange("p (g q) -> p g q", q=HD),
                        AF.Copy, bias=0.0, scale=1.0,
                    )

            # ---------------- per-head attention ----------------
            with tc.tile_pool(name="mm_ps", bufs=8, space="PSUM") as mmps:

                def out_tail(poq, h):
                    rall = cpool.tile([P, NT], F32, tag="rall", name="rall")
                    out_stage = outp.tile([P, NT, HD], F32, tag="outst", name="outst")
                    out_view = out_d[:].rearrange("(c p) (hh d) -> p c hh d", p=P, d=HD)
                    for qd in range(4):
                        nc.vector.reciprocal(
                            rall[:, qd * 4 : (qd + 1) * 4],
                            poq[qd][:, :, 0:1].rearrange("p c one -> p (c one)"),
                        )
                        for cgm in range(4):
                            cg = qd * 4 + cgm
                            if h == HEADS - 1:
                                nc.vector.tensor_scalar(
                                    out=out_stage[:, cg, :], in0=poq[qd][:, cgm, 1:NAUG],
                                    scalar1=rall[:, cg : cg + 1], scalar2=None,
                                    op0=OP.mult,
                                )
                            else:
                                nc.scalar.activation(
                                    out_stage[:, cg, :], poq[qd][:, cgm, 1:NAUG],
                                    AF.Identity, bias=0.0, scale=rall[:, cg : cg + 1],
                                )
                        nc.sync.dma_start(
                            out=out_view[:, qd * 4 : (qd + 1) * 4, h, :],
                            in_=out_stage[:, qd * 4 : (qd + 1) * 4, :],
                        )

                for h in range(HEADS):
                    poq = [
                        mmps.tile([P, 4, NAUG], F32, tag="poq", name="poq") for _ in range(4)
                    ]
                    rhs = haug[:, :, h * NAUG : (h + 1) * NAUG]
                    for pk in range(NPK):
                        # t1 = max(u_i * vq_j, q_j) per tile; AM = t1 * adjT
                        # batched over PK j-tiles per tensor_tensor.
                        # Tiles in ACT_SET use the identity max(a,q) =
                        # q + relu(a-q): ACT computes the relu part into the
                        # pack slot (freeing DVE), and the q-part is added
                        # via an extra PE accumulation adjT_t @ (q*rhs).
                        t1 = t1p.tile([P, PK, L], BF16, tag="t1", name="t1")
                        for k in range(PK):
                            t = pk * PK + k
                            if t in ACT_SET:
                                nc.scalar.activation(
                                    t1[:, k, :], ubc[:, h, :], AF.Relu,
                                    bias=nqcols[:, t, h : h + 1],
                                    scale=ecols[:, t, h : h + 1],
                                )
                            else:
                                nc.vector.tensor_scalar(
                                    out=t1[:, k, :], in0=ubc[:, h, :],
                                    scalar1=ecols[:, t, h : h + 1],
                                    scalar2=ecols[:, t, HEADS + h : HEADS + h + 1],
                                    op0=OP.mult, op1=OP.max,
                                )
                        am = amp.tile([P, PK, L], BF16, tag="am", name="am")
                        nc.vector.tensor_tensor(
                            out=am[:], in0=t1[:], in1=adjT[:, pk * PK : (pk + 1) * PK, :],
                            op=OP.mult,
                        )
                        for k in range(PK):
                            t = pk * PK + k
                            for cg in range(NT):
                                # start only on the first slice of each quad:
                                # pending-zero covers the whole PSUM bank.
                                nc.tensor.matmul(
                                    poq[cg // 4][:, cg % 4, :],
                                    am[:, k, cg * P : (cg + 1) * P], rhs[:, t, :],
                                    start=(t == 0 and cg % 4 == 0), stop=(t == NT - 1),
                                    skip_group_check=True,
                                )
                            if t in ACT_SET:
                                rhs2 = outp.tile([P, NAUG], BF16, tag="rhs2", name="rhs2")
                                nc.vector.tensor_scalar(
                                    out=rhs2[:], in0=rhs[:, t, :],
                                    scalar1=ecols[:, t, HEADS + h : HEADS + h + 1],
                                    scalar2=None, op0=OP.mult,
                                )
                                for cg in range(NT):
                                    nc.tensor.matmul(
                                        poq[cg // 4][:, cg % 4, :],
                                        adjT[:, t, cg * P : (cg + 1) * P], rhs2[:],
                                        start=False, stop=False,
                                        skip_group_check=True,
                                    )
                    out_tail(poq, h)

    nc.finalize()
    return nc


_LUT_BF16_01 = np.array([0x0000, 0x3F80], dtype=np.uint16)  # {0.0, 1.0} in bf16


def _prep_in_maps(x, adj_mask, W, att_src, att_dst):
    x = np.asarray(x, dtype=np.float32)
    W = np.asarray(W, dtype=np.float32)
    att_src = np.asarray(att_src, dtype=np.float32)
    att_dst = np.asarray(att_dst, dtype=np.float32)
    adj_u8 = np.asarray(adj_mask).view(np.uint8)

    w_bf = np.ascontiguousarray(W.astype(ml_dtypes.bfloat16))
    wT_bf = np.ascontiguousarray(W.T.astype(ml_dtypes.bfloat16))
    # attW layout: col h = att_src_h, col 4+h = att_dst_h, rows = cout
    attw = np.zeros((COUT, 2 * HEADS), dtype=np.float32)
    for h in range(HEADS):
        attw[HD * h : HD * (h + 1), h] = att_src[0, h, 0, :]
        attw[HD * h : HD * (h + 1), HEADS + h] = att_dst[0, h, 0, :]
    attw_bf = np.ascontiguousarray(attw.astype(ml_dtypes.bfloat16))

    in_maps = []
    for b in range(N_CORES):
        adjT_bf = np.ascontiguousarray(
            _LUT_BF16_01[adj_u8[b].T]
        ).view(ml_dtypes.bfloat16)
        xT_bf = np.ascontiguousarray(x[b].T.astype(ml_dtypes.bfloat16))
        in_maps.append(
            {
                "adjT": adjT_bf,
                "xT": xT_bf,
                "W": w_bf,
                "WT": wT_bf,
                "attW": attw_bf,
            }
        )
    return in_maps


def kernel(x, adj_mask, W, att_src, att_dst):
    if "nc" not in _NC_CACHE:
        _NC_CACHE["nc"] = _build()
    nc = _NC_CACHE["nc"]
    in_maps = _prep_in_maps(x, adj_mask, W, att_src, att_dst)
    res = run_bass_kernel_spmd(nc, in_maps, core_ids=list(range(N_CORES)))
    out = np.stack([res.results[b]["out"] for b in range(N_CORES)], axis=0)
    return out.astype(np.float32)


# revision 70
# speedup vs baseline: 1.0541x; 1.0029x over previous
"""DenseGAT Trainium2 kernel (8 NeuronCores, batch-parallel).

Math per (batch, head):
  h = x @ W.T ; a_src[i] = h[i]*att_src ; a_dst[j] = h[j]*att_dst
  s_ij = a_src[i] + a_dst[j] ; P = adj * exp(leakyrelu_0.2(s))
  out[i] = (P @ h)[i] / sum_j P[i,j]

Identity: exp(lrelu_0.2(s)) = p_i * q_j * max(1, u_i v_j)
with u = exp(0.8 a_src), v = exp(0.8 a_dst), q = exp(0.2 a_dst); p_i
cancels in the softmax ratio. Fold q into the j-side tensor_scalar:
  t1'[j,i] = q_j * max(1, u_i v_j) = max(u_i * e^{b_j}, e^{0.2 b_j})
(one DVE tensor_scalar at 4x: op0=mult scalar1=e^{b_j}, op1=max
 scalar2=e^{0.2 b_j}, per-partition vectors), then
  AM[j,i] = t1'[j,i] * adjT[j,i]     (DVE tensor_tensor at 2x,
                                      batched 4 j-tiles per instruction)
  out_aug[i,:] = sum_j AM[j,i] * [1 | h_j]   (PE, accumulated over j)
  out = out_aug[:,1:65] / out_aug[:,0]

Host pre-transposes layouts (adjT as bf16 {0,1}, xT, W/WT bf16, attW
assembled) so the device spends nothing on transposition/upcasting.
gpsimd tensor_tensor is ruled out: a concurrent gpsimd TT degrades DVE
tensor_scalar from 4x to ~1x via the shared SBUF ports (measured).
Each core handles one batch sample.
"""

import numpy as np
import ml_dtypes

import concourse.bass as bass
import concourse.mybir as mybir
import concourse.tile as tile
from concourse import bacc
from concourse.bass_utils import run_bass_kernel_spmd
from concourse.masks import make_identity

P = 128
B, L, CIN, COUT, HEADS = 8, 2048, 256, 256, 4
HD = COUT // HEADS          # 64
NT = L // P                 # 16 tiles along L
KB = CIN // P               # 2 chunks along cin/cout
NAUG = HD + 1               # 65 (col 0 = softmax denominator)
N_CORES = 8
PK = 4                      # j-tiles per batched tensor_tensor
NPK = NT // PK
# tiles whose max() runs on the Activation engine via q+relu(a-q); must
# avoid pack 0 (start flags) and pack 3 (extra MMs would follow the stop)
ACT_SET = {4, 5, 6, 9, 10, 11}

F32 = mybir.dt.float32
BF16 = mybir.dt.bfloat16
AF = mybir.ActivationFunctionType
OP = mybir.AluOpType

_NC_CACHE = {}


def _build():
    nc = bacc.Bacc(None, target_bir_lowering=False, debug=False)
    adjT_in = nc.declare_dram_parameter("adjT", [L, L], BF16, isOutput=False)
    xT_in = nc.declare_dram_parameter("xT", [CIN, L], BF16, isOutput=False)
    w_in = nc.declare_dram_parameter("W", [COUT, CIN], BF16, isOutput=False)
    wT_in = nc.declare_dram_parameter("WT", [CIN, COUT], BF16, isOutput=False)
    attw_in = nc.declare_dram_parameter("attW", [COUT, 2 * HEADS], BF16, isOutput=False)
    out_d = nc.declare_dram_parameter("out", [L, COUT], F32, isOutput=True)
    urows_d = nc.dram_tensor("urows_scratch", [4, L], BF16, kind="Internal")

    with tile.TileContext(nc) as tc:
        with (
            tc.tile_pool(name="const", bufs=1) as cpool,
            tc.tile_pool(name="big", bufs=1) as big,
            tc.tile_pool(name="t1p", bufs=2) as t1p,
            tc.tile_pool(name="amp", bufs=2) as amp,
            tc.tile_pool(name="outst", bufs=2) as outp,
        ):
            ident_bf = cpool.tile([P, P], BF16)
            make_identity(nc, ident_bf)


            # persistent tensors
            adjT = big.tile([P, NT, L], BF16)      # adjT[j%128, j//128, i]
            # x^T (cin on partitions), split [kb, L-half, col] so each DMA
            # writes one full dim-1/2 slot (sub-tile dep granularity)
            xT_bf = big.tile([P, KB, 2, L // 2], BF16)
            wT_bf = big.tile([P, KB, COUT], BF16)  # W^T (cin on partitions)
            w_nat = big.tile([P, KB, CIN], BF16)   # W natural (cout on part)
            haug = big.tile([P, NT, HEADS * NAUG], BF16)  # [1|h0][1|h1][1|h2][1|h3]
            urows = big.tile([4, L], BF16)         # exp(0.8 a_src_h) rows
            evq = big.tile([4, L], BF16)           # e^{b_h} rows
            eq = big.tile([4, L], BF16)            # e^{0.2 b_h} rows
            ecols = big.tile([P, NT, 8], F32)      # transposed: vq_h | q_h columns
            nqcols = big.tile([P, NT, 4], F32)     # -q_h columns (ACT relu bias)
            ubc = big.tile([P, HEADS, L], BF16)    # exp(0.8 a_src_h) bcast, all heads

            # ---------------- prep ----------------
            with (
                tc.tile_pool(name="big2", bufs=1) as big2,
                tc.tile_pool(name="h_ps", bufs=2, space="PSUM") as hps,
                tc.tile_pool(name="sc_ps", bufs=1, space="PSUM") as scps,
                tc.tile_pool(name="tr_ps", bufs=2, space="PSUM") as trps,
            ):
                attW = big2.tile([P, KB, 2 * HEADS], BF16)     # [cout, 8]
                attc_bf = big2.tile([P, KB, 2 * HEADS], BF16)  # [cin, 8]

                # DMAs. Two hardware queues: sync gets xT's first L-half
                # (both cin chunks — unblocks scores/h for L-half 0 early)
                # then the adjT stream; scalar(Activation) gets the small
                # weights + xT's second L-half, later urows/ubcast.
                LH = L // 2
                nc.sync.dma_start(
                    out=xT_bf[:, 0, 0, :], in_=xT_in[0:P, 0:LH]
                )
                nc.sync.dma_start(
                    out=xT_bf[:, 1, 0, :], in_=xT_in[P : 2 * P, 0:LH]
                )
                nc.scalar.dma_start(
                    out=attW[:], in_=attw_in[:].rearrange("(kb p) c -> p kb c", p=P)
                )
                nc.scalar.dma_start(
                    out=w_nat[:], in_=w_in[:].rearrange("(kb p) c -> p kb c", p=P)
                )
                nc.scalar.dma_start(
                    out=xT_bf[:, 0, 1, :], in_=xT_in[0:P, LH:L]
                )
                nc.scalar.dma_start(
                    out=xT_bf[:, 1, 1, :], in_=xT_in[P : 2 * P, LH:L]
                )
                nc.scalar.dma_start(
                    out=wT_bf[:], in_=wT_in[:].rearrange("(kb p) o -> p kb o", p=P)
                )
                for t in range(NT):
                    nc.sync.dma_start(
                        out=adjT[:, t, :], in_=adjT_in[t * P : (t + 1) * P, :]
                    )

                # attc = W^T @ attW  (tiny)
                for mb in range(KB):
                    ap_ps = trps.tile([P, 2 * HEADS], F32, tag="tr")
                    for cb in range(KB):
                        nc.tensor.matmul(
                            ap_ps[:], w_nat[:, cb, mb * P : (mb + 1) * P], attW[:, cb, :],
                            start=(cb == 0), stop=(cb == KB - 1),
                        )
                    nc.scalar.activation(attc_bf[:, mb, :], ap_ps[:], AF.Copy, bias=0.0, scale=1.0)

                # scores: src rows and dst rows in separate PSUM tiles
                # (PSUM reads must start at partition 0), split in L-halves
                # to fit banks; exps applied straight from PSUM.
                #   urows = e^{0.8 a_src}; erows = [e^{b} | e^{0.2 b}]
                for h2 in range(2):
                    sl = slice(h2 * 1024, (h2 + 1) * 1024)
                    asrc_ps = scps.tile([4, 2, 512], F32, tag="sc_src")
                    adst_ps = scps.tile([4, 2, 512], F32, tag="sc_dst")
                    for nb in range(2):
                        for kb in range(KB):
                            nc.tensor.matmul(
                                asrc_ps[:, nb, :], attc_bf[:, kb, 0:HEADS],
                                xT_bf[:, kb, h2, nb * 512 : (nb + 1) * 512],
                                start=(kb == 0), stop=(kb == KB - 1),
                            )
                    for nb in range(2):
                        for kb in range(KB):
                            nc.tensor.matmul(
                                adst_ps[:, nb, :], attc_bf[:, kb, HEADS : 2 * HEADS],
                                xT_bf[:, kb, h2, nb * 512 : (nb + 1) * 512],
                                start=(kb == 0), stop=(kb == KB - 1),
                            )
                    nc.scalar.activation(
                        urows[:, sl].rearrange("p (nb c) -> p nb c", c=512),
                        asrc_ps[:], AF.Exp, bias=0.0, scale=0.8,
                    )
                    nc.scalar.activation(
                        evq[:, sl].rearrange("p (nb c) -> p nb c", c=512),
                        adst_ps[:], AF.Exp, bias=0.0, scale=1.0,
                    )
                    nc.scalar.activation(
                        eq[:, sl].rearrange("p (nb c) -> p nb c", c=512),
                        adst_ps[:], AF.Exp, bias=0.0, scale=0.2,
                    )
                nc.scalar.dma_start(out=urows_d[:], in_=urows[:])

                # transposed per-j columns: ecols[:,t,0:4]=vq, 4:8=q
                for t in range(NT):
                    ecp = trps.tile([P, 8], BF16, tag="tr")
                    nc.tensor.transpose(
                        ecp[:, 0:4], evq[:, t * P : (t + 1) * P], ident_bf[0:4, 0:4]
                    )
                    nc.tensor.transpose(
                        ecp[:, 4:8], eq[:, t * P : (t + 1) * P], ident_bf[0:4, 0:4]
                    )
                    nc.vector.tensor_copy(ecols[:, t, :], ecp[:])
                nc.scalar.activation(
                    nqcols[:], ecols[:, :, 4:8], AF.Copy, bias=0.0, scale=-1.0
                )

                # ubcast for all heads upfront (overlaps head-0 compute)
                for h in range(HEADS):
                    nc.scalar.dma_start(
                        out=ubc[:, h, :], in_=urows_d[h : h + 1, :].to_broadcast((P, L))
                    )

                # haug = [1 | h] per head; ones first so h-evac can overwrite
                for h in range(HEADS):
                    nc.vector.memset(
                        haug[:, :, h * NAUG : h * NAUG + 1].rearrange("p t one -> p (t one)"),
                        1.0,
                    )
                # h = x @ W.T, written strided into the 4 head slots
                for c in range(NT):
                    hp = hps.tile([P, COUT], F32, tag="hp")
                    for kb in range(KB):
                        nc.tensor.matmul(
                            hp[:], xT_bf[:, kb, c // 8, (c % 8) * P : (c % 8 + 1) * P],
                            wT_bf[:, kb, :],
                            start=(kb == 0), stop=(kb == KB - 1),
                        )
                    dst = haug[:, c, :].rearrange("p (g q) -> p g q", q=NAUG)[:, :, 1:NAUG]
                    nc.scalar.activation(
                        dst, hp[:].rearrange("p (g q) -> p g q", q=HD),
                        AF.Copy, bias=0.0, scale=1.0,
                    )

            # ---------------- per-head attention ----------------
            with tc.tile_pool(name="mm_ps", bufs=8, space="PSUM") as mmps:

                def out_tail(poq, h):
                    rall = cpool.tile([P, NT], F32, tag="rall", name="rall")
                    out_stage = outp.tile([P, NT, HD], F32, tag="outst", name="outst")
                    out_view = out_d[:].rearrange("(c p) (hh d) -> p c hh d", p=P, d=HD)
                    for qd in range(4):
                        nc.vector.reciprocal(
                            rall[:, qd * 4 : (qd + 1) * 4],
                            poq[qd][:, :, 0:1].rearrange("p c one -> p (c one)"),
                        )
                        for cgm in range(4):
                            cg = qd * 4 + cgm
                            if h == HEADS - 1:
                                nc.vector.tensor_scalar(
                                    out=out_stage[:, cg, :], in0=poq[qd][:, cgm, 1:NAUG],
                                    scalar1=rall[:, cg : cg + 1], scalar2=None,
                                    op0=OP.mult,
                                )
                            else:
                                nc.scalar.activation(
                                    out_stage[:, cg, :], poq[qd][:, cgm, 1:NAUG],
                                    AF.Identity, bias=0.0, scale=rall[:, cg : cg + 1],
                                )
                        nc.sync.dma_start(
                            out=out_view[:, qd * 4 : (qd + 1) * 4, h, :],
                            in_=out_stage[:, qd * 4 : (qd + 1) * 4, :],
                        )

                for h in range(HEADS):
                    poq = [
                        mmps.tile([P, 4, NAUG], F32, tag="poq", name="poq") for _ in range(4)
                    ]
                    rhs = haug[:, :, h * NAUG : (h + 1) * NAUG]
                    for pk in range(NPK):
                        # t1 = max(u_i * vq_j, q_j) per tile; AM = t1 * adjT
                        # batched over PK j-tiles per tensor_tensor.
                        # Tiles in ACT_SET use the identity max(a,q) =
                        # q + relu(a-q): ACT computes the relu part into the
                        # pack slot (freeing DVE), and the q-part is added
                        # via an extra PE accumulation adjT_t @ (q*rhs).
                        t1 = t1p.tile([P, PK, L], BF16, tag="t1", name="t1")
                        for k in range(PK):
                            t = pk * PK + k
                            if t in ACT_SET:
                                nc.scalar.activation(
                                    t1[:, k, :], ubc[:, h, :], AF.Relu,
                                    bias=nqcols[:, t, h : h + 1],
                                    scale=ecols[:, t, h : h + 1],
                                )
                            else:
                                nc.vector.tensor_scalar(
                                    out=t1[:, k, :], in0=ubc[:, h, :],
                                    scalar1=ecols[:, t, h : h + 1],
                                    scalar2=ecols[:, t, HEADS + h : HEADS + h + 1],
                                    op0=OP.mult, op1=OP.max,
                                )
                        am = amp.tile([P, PK, L], BF16, tag="am", name="am")
                        nc.vector.tensor_tensor(
                            out=am[:], in0=t1[:], in1=adjT[:, pk * PK : (pk + 1) * PK, :],
                            op=OP.mult,
                        )
                        for k in range(PK):
                            t = pk * PK + k
                            for cg in range(NT):
                                # start only on the first slice of each quad:
                                # pending-zero covers the whole PSUM bank.
                                nc.tensor.matmul(
                                    poq[cg // 4][:, cg % 4, :],
                                    am[:, k, cg * P : (cg + 1) * P], rhs[:, t, :],
                                    start=(t == 0 and cg % 4 == 0), stop=(t == NT - 1),
                                    skip_group_check=True,
                                )
                            if t in ACT_SET:
                                rhs2 = outp.tile([P, NAUG], BF16, tag="rhs2", name="rhs2")
                                nc.vector.tensor_scalar(
                                    out=rhs2[:], in0=rhs[:, t, :],
                                    scalar1=ecols[:, t, HEADS + h : HEADS + h + 1],
                                    scalar2=None, op0=OP.mult,
                                )
                                for cg in range(NT):
                                    nc.tensor.matmul(
                                        poq[cg // 4][:, cg % 4, :],
                                        adjT[:, t, cg * P : (cg + 1) * P], rhs2[:],
                                        start=False, stop=False,
                                        skip_group_check=True,
                                    )
                    out_tail(poq, h)

    nc.finalize()
    return nc


_LUT_BF16_01 = np.array([0x0000, 0x3F80], dtype=np.uint16)  # {0.0, 1.0} in bf16


def _prep_in_maps(x, adj_mask, W, att_src, att_dst):
    x = np.asarray(x, dtype=np.float32)
    W = np.asarray(W, dtype=np.float32)
    att_src = np.asarray(att_src, dtype=np.float32)
    att_dst = np.asarray(att_dst, dtype=np.float32)
    adj_u8 = np.asarray(adj_mask).view(np.uint8)

    w_bf = np.ascontiguousarray(W.astype(ml_dtypes.bfloat16))
    wT_bf = np.ascontiguousarray(W.T.astype(ml_dtypes.bfloat16))
    # attW layout: col h = att_src_h, col 4+h = att_dst_h, rows = cout
    attw = np.zeros((COUT, 2 * HEADS), dtype=np.float32)
    for h in range(HEADS):
        attw[HD * h : HD * (h + 1), h] = att_src[0, h, 0, :]
        attw[HD * h : HD * (h + 1), HEADS + h] = att_dst[0, h, 0, :]
    attw_bf = np.ascontiguousarray(attw.astype(ml_dtypes.bfloat16))

    in_maps = []
    for b in range(N_CORES):
        adjT_bf = np.ascontiguousarray(
            _LUT_BF16_01[adj_u8[b].T]
        ).view(ml_dtypes.bfloat16)
        xT_bf = np.ascontiguousarray(x[b].T.astype(ml_dtypes.bfloat16))
        in_maps.append(
            {
                "adjT": adjT_bf,
                "xT": xT_bf,
                "W": w_bf,
                "WT": wT_bf,
                "attW": attw_bf,
            }
        )
    return in_maps


def kernel(x, adj_mask, W, att_src, att_dst):
    if "nc" not in _NC_CACHE:
        _NC_CACHE["nc"] = _build()
    nc = _NC_CACHE["nc"]
    in_maps = _prep_in_maps(x, adj_mask, W, att_src, att_dst)
    res = run_bass_kernel_spmd(nc, in_maps, core_ids=list(range(N_CORES)))
    out = np.stack([res.results[b]["out"] for b in range(N_CORES)], axis=0)
    return out.astype(np.float32)
